# revision 27
# baseline (speedup 1.0000x reference)
"""DKVMN forward kernel for 8 Trainium2 NeuronCores (Bass/Tile) — v2.

Design (replaces the tensor_tensor_scan/slot design):
 - Data-parallel over batch: core c handles batches [c*32, (c+1)*32).
 - State layout: p = b_local*4 + kq (128 partitions), free = (k2=64, v=128);
   M[p, k2*128+v] = M_state[b][v, kq*64+k2], bf16.
 - Device-built DRAM table DUtab[(x, tab, q)] of quarter-rows (8192 core
   elems + 128 tail): D = 1 - w[item(x)] (x) e[x] (outer, k-major/v-inner),
   U = w (x) a.  D-rows carry w[item] in the tail for the read-multiply.
 - Per chunk (C=2 steps): two dma_gathers (16.6KB rows, ~8.5MB/chunk) fetch
   D/U/w for 2 steps; per step 4 packed DVE ops (all 2x-mode eligible):
     RT = M * w_bcast ; r = reduce_v(RT) ; M = M * D ; M = M + U
 - reads stored bf16 token-major [tok, k]; head uses dma_start_transpose
   (xbar) to get [k, tok] tiles, PE matmuls for fW1@r and p_W.f, KF via
   transposed dma_gather.  No strided 4-byte DMA anywhere.
"""
import os
import sys
import numpy as np
import ml_dtypes

sys.path.insert(0, '/opt/trn_rl_repo')

import concourse.bass as bass          # noqa: E402
import concourse.bacc as bacc          # noqa: E402
import concourse.mybir as mybir        # noqa: E402
from concourse.tile import TileContext # noqa: E402
from concourse.bass_utils import run_bass_kernel_spmd  # noqa: E402

F32 = mybir.dt.float32
BF16 = mybir.dt.bfloat16
I16 = mybir.dt.int16
ALU = mybir.AluOpType
ACTF = mybir.ActivationFunctionType
BF = ml_dtypes.bfloat16

NUM_ITEM = 2000
DK = 256          # key dim
DV = 128          # memory slots (v)
B, T = 256, 512
NC = 8
BL = B // NC      # 32 local batches
KSUB = 4          # k quarters on partitions
K2 = DK // KSUB   # 64
P = BL * KSUB     # 128 partitions: p = kq*32 + b (kq-major)
SEG = K2 * DV     # 8192 state cells per partition
C = 2             # steps per chunk
NCH = T // C      # 256 chunks
NIT = 2048        # padded item count
NX = 4096         # padded x count (e/a tables)
NXR = 4000        # real x count (D/U tables)
ROW = SEG + DV    # 8320 elems per D quarter-row (core + w tail)
TOK = BL * T      # 16384 tokens per core
TH = TOK // 2     # head token-half

_cache = {}
LAST_RESULT = None  # BassKernelResults of the most recent run (for test harness)


def _wrap16(vals):
    """int index array [n] -> [128, n/16] wrapped-in-16 + replicated x8."""
    vals = np.asarray(vals, np.int64)
    n = len(vals)
    assert n % 16 == 0
    a = np.zeros((16, n // 16), np.int16)
    a[np.arange(n) % 16, np.arange(n) // 16] = vals
    return np.tile(a, (8, 1))


def build_program():
    nc = bacc.Bacc(None, target_bir_lowering=False, debug=False)

    # ---- external inputs (host-prepped) ----
    kT = nc.dram_tensor("kT", [DK, NIT], BF16, kind="ExternalInput")        # k_emb^T padded
    vT = nc.dram_tensor("vT", [DK, NX], BF16, kind="ExternalInput")         # v_emb^T padded
    MkT = nc.dram_tensor("MkT", [DK, DV], BF16, kind="ExternalInput")       # Mk^T
    eaWT = nc.dram_tensor("eaWT", [DK, 2 * DK], BF16, kind="ExternalInput")  # [e_W^T | a_W^T]
    fW2T = nc.dram_tensor("fW2T", [DK, DK], BF16, kind="ExternalInput")     # f_W[:,256:]^T
    fW1T = nc.dram_tensor("fW1T", [DK, DK], BF16, kind="ExternalInput")     # f_W[:,:256]^T
    onesf = nc.dram_tensor("onesf", [1, 128], F32, kind="ExternalInput")
    eab = nc.dram_tensor("eab", [1, 2 * DK], F32, kind="ExternalInput")     # [e_b | a_b]
    fbcol = nc.dram_tensor("fbcol", [128, 2], F32, kind="ExternalInput")    # f_b by ko-half
    pwcol = nc.dram_tensor("pwcol", [128, 2], BF16, kind="ExternalInput")   # p_W by ko-half
    pbt = nc.dram_tensor("pbt", [1, 1], F32, kind="ExternalInput")
    m0 = nc.dram_tensor("m0", [P, SEG], BF16, kind="ExternalInput")
    cidx = nc.dram_tensor("cidx", [P, NCH * 16], I16, kind="ExternalInput")
    wxidx = nc.dram_tensor("wxidx", [P, NX // 16], I16, kind="ExternalInput")
    kfidx = nc.dram_tensor("kfidx", [P, TOK // 16], I16, kind="ExternalInput")

    pred = nc.dram_tensor("pred", [1, TOK], F32, kind="ExternalOutput")

    # ---- DRAM scratch ----
    WtabD = nc.dram_tensor("WtabD", [NIT, DV], BF16)      # softmax rows (item)
    KFtabD = nc.dram_tensor("KFtabD", [NIT, DK], BF16)    # k_emb @ fW2^T rows
    Dtab = nc.dram_tensor("Dtab", [NXR * 4, ROW], BF16)   # (x, q): 1-w(x)e + w tail
    Utab = nc.dram_tensor("Utab", [NXR * 4, SEG], BF16)   # (x, q): w(x)a
    rT_d2 = nc.dram_tensor("rT_d2", [TOK, DK], BF16)      # reads, token-major

    with TileContext(nc) as tc:
        # ================= stage 1a: w / e / a / kf tables =================
        with tc.tile_pool(name="keep", bufs=1) as kp:
            ea_bf = kp.tile([128, 32, 2 * DK], BF16, tag="eabf")   # sig/tanh rows (x)
            wx_sb = kp.tile([128, 32, DV], BF16, tag="wx")         # w rows by x
            with (
                tc.tile_pool(name="wpool", bufs=1) as wp,
                tc.tile_pool(name="tpool", bufs=1) as tp,
                tc.tile_pool(name="pspool", bufs=2, space="PSUM") as pp,
            ):
                kT_s = [wp.tile([128, NIT], BF16, tag=f"kt{i}", name=f"kt{i}") for i in range(2)]
                vT_s = [wp.tile([128, NX], BF16, tag=f"vt{i}", name=f"vt{i}") for i in range(2)]
                MkT_s = [wp.tile([128, DV], BF16, tag=f"mk{i}", name=f"mk{i}") for i in range(2)]
                eaWT_s = [wp.tile([128, 2 * DK], BF16, tag=f"ea{i}", name=f"eaw{i}") for i in range(2)]
                fW2T_s = [wp.tile([128, DK], BF16, tag=f"f2{i}", name=f"f2{i}") for i in range(2)]
                onesf_s = wp.tile([1, 128], F32, tag="onf")
                eab_s = wp.tile([1, 2 * DK], F32, tag="eb")
                for i in range(2):
                    nc.sync.dma_start(kT_s[i][:], kT[128 * i:128 * (i + 1), :])
                    nc.sync.dma_start(vT_s[i][:], vT[128 * i:128 * (i + 1), :])
                    nc.sync.dma_start(MkT_s[i][:], MkT[128 * i:128 * (i + 1), :])
                    nc.sync.dma_start(eaWT_s[i][:], eaWT[128 * i:128 * (i + 1), :])
                    nc.sync.dma_start(fW2T_s[i][:], fW2T[128 * i:128 * (i + 1), :])
                nc.sync.dma_start(onesf_s[:], onesf[:])
                nc.sync.dma_start(eab_s[:], eab[:])

                # --- softmax(k_emb @ Mk^T) rows -> WtabD (bf16) ---
                wexp = tp.tile([128, 16, DV], F32, tag="wexp")
                for it in range(16):
                    ps = pp.tile([128, DV], F32, tag="ps_w")
                    sl = slice(128 * it, 128 * (it + 1))
                    nc.tensor.matmul(out=ps[:], lhsT=kT_s[0][:, sl], rhs=MkT_s[0][:],
                                     start=True, stop=False)
                    nc.tensor.matmul(out=ps[:], lhsT=kT_s[1][:, sl], rhs=MkT_s[1][:],
                                     start=False, stop=True)
                    nc.scalar.activation(out=wexp[:, it, :], in_=ps[:], func=ACTF.Exp)
                zs = tp.tile([128, 16], F32, tag="zs")
                nc.vector.tensor_reduce(out=zs[:], in_=wexp[:], axis=mybir.AxisListType.X,
                                        op=ALU.add)
                zr = tp.tile([128, 16], F32, tag="zr")
                nc.vector.reciprocal(out=zr[:], in_=zs[:])
                wbf = tp.tile([128, 16, DV], BF16, tag="wbf")
                nc.vector.tensor_tensor(
                    out=wbf[:], in0=wexp[:],
                    in1=zr[:].unsqueeze(2).to_broadcast([128, 16, DV]), op=ALU.mult)
                nc.sync.dma_start(
                    WtabD[:].rearrange("(it p) v -> p it v", p=128), wbf[:])

                # --- sigmoid/tanh(v_emb @ [eW|aW]^T + [eb|ab]) rows (by x) ---
                for it in range(32):
                    ps = pp.tile([128, 2 * DK], F32, tag="ps_ea")
                    sl = slice(128 * it, 128 * (it + 1))
                    nc.tensor.matmul(out=ps[:], lhsT=vT_s[0][:, sl], rhs=eaWT_s[0][:],
                                     start=True, stop=False)
                    nc.tensor.matmul(out=ps[:], lhsT=vT_s[1][:, sl], rhs=eaWT_s[1][:],
                                     start=False, stop=False)
                    nc.tensor.matmul(out=ps[:], lhsT=onesf_s[:], rhs=eab_s[:],
                                     start=False, stop=True)
                    nc.scalar.activation(out=ea_bf[:, it, 0:DK], in_=ps[:, 0:DK],
                                         func=ACTF.Sigmoid)
                    nc.scalar.activation(out=ea_bf[:, it, DK:2 * DK], in_=ps[:, DK:2 * DK],
                                         func=ACTF.Tanh)

                # --- KFtabD: k_emb @ fW2^T (NO f_b; added later via ACT bias) ---
                kfbf = tp.tile([128, 16, DK], BF16, tag="kfbf")
                for it in range(16):
                    ps = pp.tile([128, DK], F32, tag="ps_kf")
                    sl = slice(128 * it, 128 * (it + 1))
                    nc.tensor.matmul(out=ps[:], lhsT=kT_s[0][:, sl], rhs=fW2T_s[0][:],
                                     start=True, stop=False)
                    nc.tensor.matmul(out=ps[:], lhsT=kT_s[1][:, sl], rhs=fW2T_s[1][:],
                                     start=False, stop=True)
                    nc.scalar.copy(out=kfbf[:, it, :], in_=ps[:])
                nc.sync.dma_start(
                    KFtabD[:].rearrange("(it p) c -> p it c", p=128), kfbf[:])

                # w rows re-fetched x-aligned (item(x) = x mod 2000)
                wxi_s = tp.tile([P, NX // 16], I16, tag="wxi")
                nc.sync.dma_start(wxi_s[:], wxidx[:])
                for gq in range(4):
                    nc.gpsimd.dma_gather(
                        wx_sb[:, 8 * gq:8 * (gq + 1), :], WtabD[:],
                        wxi_s[:, 64 * gq:64 * (gq + 1)], NX // 4, NX // 4, DV)

            # ================= stage 1b: D/U table build =================
            D4 = Dtab[:].rearrange("(x q) e -> x (q e)", q=KSUB)
            U4 = Utab[:].rearrange("(x q) e -> x q e", q=KSUB)
            with tc.tile_pool(name="bpool", bufs=2) as bp:
                for g in range(32):
                    npart = min(128, NXR - 128 * g)   # last tile covers 32 x's
                    xsl = slice(128 * g, 128 * g + npart)
                    for tab in range(2):
                        BT = bp.tile([128, KSUB, ROW], BF16, tag="bt")
                        core = BT[:, :, 0:SEG].rearrange(
                            "p q (k2 v) -> p q k2 v", v=DV)      # [p, 4, 64, 128]
                        col = slice(0, DK) if tab == 0 else slice(DK, 2 * DK)
                        e4 = ea_bf[:, g, col].rearrange(
                            "p (q k2) -> p q k2", q=KSUB).unsqueeze(3) \
                            .to_broadcast([128, KSUB, K2, DV])
                        w4 = wx_sb[:, g, :].unsqueeze(1).unsqueeze(2) \
                            .to_broadcast([128, KSUB, K2, DV])
                        nc.vector.tensor_tensor(out=core, in0=e4, in1=w4,
                                                op=ALU.mult)
                        if tab == 0:
                            nc.vector.tensor_scalar(
                                out=core, in0=core, scalar1=-1.0, scalar2=1.0,
                                op0=ALU.mult, op1=ALU.add)
                            nc.vector.tensor_copy(
                                out=BT[:, :, SEG:ROW],
                                in_=wx_sb[:, g, :].unsqueeze(1).to_broadcast(
                                    [128, KSUB, DV]))
                            nc.sync.dma_start(
                                D4[xsl, :],
                                BT[0:npart].rearrange("p q e -> p (q e)"))
                        else:
                            nc.sync.dma_start(
                                U4[xsl], BT[0:npart, :, 0:SEG])

        # ================= stage 2: recurrence =================
        with (
            tc.tile_pool(name="scst", bufs=1) as st,
            tc.tile_pool(name="scg", bufs=2) as sg,
            tc.tile_pool(name="rp", bufs=2) as rp,
        ):
            Mt = st.tile([P, SEG], BF16, tag="M")
            RT = st.tile([P, SEG], BF16, tag="RT")
            cidx_sb = st.tile([P, NCH, 16], I16, tag="cix")
            nc.sync.dma_start(Mt[:], m0[:])
            nc.sync.dma_start(cidx_sb[:].rearrange("p a b -> p (a b)"), cidx[:])

            M3 = Mt[:].rearrange("p (k v) -> p k v", v=DV)
            RT3 = RT[:].rearrange("p (k v) -> p k v", v=DV)
            R2 = rT_d2[:].rearrange("(b t) k -> b t k", b=BL)

            def fetch(ch):
                Dg = sg.tile([P, C, ROW], BF16, tag="dg", name="Dg")
                Ug = sg.tile([P, C, SEG], BF16, tag="ug", name="Ug")
                nc.gpsimd.dma_gather(Dg[:], Dtab[:],
                                     cidx_sb[:, ch, :], C * P, C * P, ROW)
                nc.gpsimd.dma_gather(Ug[:], Utab[:],
                                     cidx_sb[:, ch, :], C * P, C * P, SEG)
                return Dg, Ug

            nxt = fetch(0)
            for ch in range(NCH):
                Dg, Ug = nxt
                if ch + 1 < NCH:
                    nxt = fetch(ch + 1)
                rTf = rp.tile([P, C, K2], F32, tag="rtf")
                rTb = rp.tile([P, C, K2], BF16, tag="rtb")
                RT1 = rp.tile([P, SEG], BF16, tag="rt1")
                RT13 = RT1[:].rearrange("p (k v) -> p k v", v=DV)
                for s in range(C):
                    w_ap = Dg[:, s, SEG:ROW].unsqueeze(1).to_broadcast([P, K2, DV])
                    if s == 0:
                        nc.vector.tensor_tensor(out=RT3, in0=M3, in1=w_ap,
                                                op=ALU.mult)
                        nc.vector.tensor_reduce(out=rTf[:, s, :], in_=RT3,
                                                axis=mybir.AxisListType.X,
                                                op=ALU.add)
                    else:
                        nc.vector.tensor_tensor(out=RT13, in0=M3, in1=w_ap,
                                                op=ALU.mult)
                        wd = DV // 2
                        while wd >= 1:
                            dst = (rTf[:, s, :].unsqueeze(2)
                                   if wd == 1 else RT13[:, :, 0:wd])
                            nc.gpsimd.tensor_tensor(
                                out=dst, in0=RT13[:, :, 0:wd],
                                in1=RT13[:, :, wd:2 * wd], op=ALU.add)
                            wd //= 2
                    nc.vector.tensor_tensor(out=Mt[:], in0=Mt[:],
                                            in1=Dg[:, s, 0:SEG], op=ALU.mult)
                    nc.vector.tensor_tensor(out=Mt[:], in0=Mt[:],
                                            in1=Ug[:, s, :], op=ALU.add)
                nc.scalar.copy(out=rTb[:], in_=rTf[:])
                for kq in range(KSUB):
                    nc.sync.dma_start(
                        R2[:, C * ch:C * (ch + 1), K2 * kq:K2 * (kq + 1)],
                        rTb[BL * kq:BL * (kq + 1), :, :])

        # ================= stage 3: head =================
        with (
            tc.tile_pool(name="hw", bufs=1) as hw,
            tc.tile_pool(name="hp", bufs=1) as hpool,
            tc.tile_pool(name="hps", bufs=4, space="PSUM") as hps,
            tc.tile_pool(name="hps2", bufs=4, space="PSUM") as hps2,
        ):
            f1 = [hw.tile([128, DK], BF16, tag=f"f1{h}", name=f"f1{h}") for h in range(2)]
            for h in range(2):
                nc.sync.dma_start(f1[h][:], fW1T[128 * h:128 * (h + 1), :])
            pw_s = hw.tile([128, 2], BF16, tag="pw")
            fb_s = hw.tile([128, 2], F32, tag="fb")
            pb_s = hw.tile([1, 1], F32, tag="pb")
            kfi_s = hw.tile([P, TOK // 16], I16, tag="kfi")
            pred_sb = hw.tile([1, TOK], F32, tag="prd")
            nc.sync.dma_start(pw_s[:], pwcol[:])
            nc.sync.dma_start(fb_s[:], fbcol[:])
            nc.sync.dma_start(pb_s[:], pbt[:])
            nc.sync.dma_start(kfi_s[:], kfidx[:])

            for th in range(2):
                toks = slice(th * TH, (th + 1) * TH)
                rq = [hpool.tile([128, TH], BF16, tag=f"rq{h}", name=f"rq{h}") for h in range(2)]
                for h in range(2):
                    nc.sync.dma_start_transpose(
                        rq[h][:], rT_d2[toks, 128 * h:128 * (h + 1)])
                kfT = hpool.tile([128, TH // 512, 2, 512], BF16, tag="kft")
                for gq in range(TH // 512):
                    nc.gpsimd.dma_gather(
                        kfT[:, gq, :, :], KFtabD[:],
                        kfi_s[:, th * (TH // 16) + 32 * gq:
                              th * (TH // 16) + 32 * (gq + 1)],
                        512, 512, DK, transpose=True)
                fq = hpool.tile([128, 2, TH], BF16, tag="fq")
                for m in range(2):
                    for n in range(TH // 512):
                        nsl = slice(512 * n, 512 * (n + 1))
                        ps = hps.tile([128, 512], F32, tag="psg")
                        nc.tensor.matmul(out=ps[:], lhsT=f1[0][:, 128 * m:128 * (m + 1)],
                                         rhs=rq[0][:, nsl], start=True, stop=False)
                        nc.tensor.matmul(out=ps[:], lhsT=f1[1][:, 128 * m:128 * (m + 1)],
                                         rhs=rq[1][:, nsl], start=False, stop=True)
                        nc.vector.tensor_tensor(out=fq[:, m, nsl], in0=ps[:],
                                                in1=kfT[:, n, m, :], op=ALU.add)
                    nc.scalar.activation(out=fq[:, m, :], in_=fq[:, m, :],
                                         func=ACTF.Tanh, bias=fb_s[:, m:m + 1])
                for n in range(TH // 512):
                    nsl = slice(512 * n, 512 * (n + 1))
                    ps2 = hps2.tile([1, 512], F32, tag="psp")
                    nc.tensor.matmul(out=ps2[:], lhsT=pw_s[:, 0:1],
                                     rhs=fq[:, 0, nsl], start=True, stop=False)
                    nc.tensor.matmul(out=ps2[:], lhsT=pw_s[:, 1:2],
                                     rhs=fq[:, 1, nsl], start=False, stop=True)
                    nc.scalar.activation(out=pred_sb[:, th * TH + 512 * n:
                                                     th * TH + 512 * (n + 1)],
                                         in_=ps2[:], func=ACTF.Sigmoid,
                                         bias=pb_s[:])
            nc.sync.dma_start(pred[:], pred_sb[:])

    nc.finalize()
    return nc


def _host_shared(k_emb, v_emb, Mk, Mv0, e_W, e_b, a_W, a_b, f_W, f_b, p_W, p_b):
    pad_k = np.zeros((NIT, DK), np.float32)
    pad_k[:NUM_ITEM] = k_emb
    pad_v = np.zeros((NX, DK), np.float32)
    pad_v[:2 * NUM_ITEM] = v_emb
    fbcol = np.zeros((128, 2), np.float32)
    fbcol[:, 0] = f_b[:128]
    fbcol[:, 1] = f_b[128:]
    pwcol = np.zeros((128, 2), np.float32)
    pwcol[:, 0] = p_W[0, :128]
    pwcol[:, 1] = p_W[0, 128:]
    # m0[p, k2*128+v] = Mv0[v, kq*64+k2] with p = kq*32 + b
    kq = np.arange(P) // BL
    k2i, vi = np.meshgrid(np.arange(K2), np.arange(DV), indexing="ij")
    m0 = Mv0.T[(kq[:, None, None] * K2 + k2i[None]), vi[None]].reshape(P, SEG)
    # wxidx: item(x) for x in [0, NX)
    items = np.zeros(NX, np.int64)
    items[:2 * NUM_ITEM] = np.arange(2 * NUM_ITEM) % NUM_ITEM
    return {
        "kT": np.ascontiguousarray(pad_k.T).astype(BF),
        "vT": np.ascontiguousarray(pad_v.T).astype(BF),
        "MkT": np.ascontiguousarray(Mk.T).astype(BF),
        "eaWT": np.ascontiguousarray(np.concatenate([e_W.T, a_W.T], axis=1)).astype(BF),
        "fW2T": np.ascontiguousarray(f_W[:, DK:].T).astype(BF),
        "fW1T": np.ascontiguousarray(f_W[:, :DK].T).astype(BF),
        "onesf": np.ones((1, 128), np.float32),
        "eab": np.concatenate([e_b, a_b])[None, :].astype(np.float32),
        "fbcol": fbcol,
        "pwcol": pwcol.astype(BF),
        "pbt": np.array([[float(p_b[0])]], np.float32),
        "m0": m0.astype(BF),
        "wxidx": _wrap16(items),
    }


def _host_core(item_c, x_c):
    """Per-core index tensors. item_c/x_c: [BL, T] int64. p = kq*32 + b."""
    b = np.arange(P) % BL
    kq = np.arange(P) // BL
    xp = x_c[b[None, :], np.arange(T)[:, None]]       # [T, P]
    rows = xp * KSUB + kq[None, :]                     # [T, P]
    cidx = np.zeros((P, NCH, 16), np.int16)
    for ch in range(NCH):
        cidx[:, ch, :] = _wrap16(rows[C * ch:C * (ch + 1), :].reshape(-1))
    return {
        "cidx": cidx.reshape(P, NCH * 16),
        "kfidx": _wrap16(item_c.reshape(-1)),
    }


def kernel(**inputs):
    inputs = {k: np.asarray(v) for k, v in inputs.items()}
    item = inputs["item_seq"].astype(np.int64)
    corr = inputs["correct_seq"].astype(np.int64)
    x = item + NUM_ITEM * corr

    if "nc" not in _cache:
        _cache["nc"] = build_program()
    nc = _cache["nc"]

    shared = _host_shared(
        inputs["k_emb"].astype(np.float32), inputs["v_emb"].astype(np.float32),
        inputs["Mk"].astype(np.float32), inputs["Mv0"].astype(np.float32),
        inputs["e_W"].astype(np.float32), inputs["e_b"].astype(np.float32),
        inputs["a_W"].astype(np.float32), inputs["a_b"].astype(np.float32),
        inputs["f_W"].astype(np.float32), inputs["f_b"].astype(np.float32),
        inputs["p_W"].astype(np.float32), inputs["p_b"].astype(np.float32))

    in_maps = []
    for c in range(NC):
        sl = slice(c * BL, (c + 1) * BL)
        m = dict(shared)
        m.update(_host_core(item[sl], x[sl]))
        in_maps.append(m)

    tdir = os.environ.get("BASS_KERNEL_TRACE_DIR")
    res = run_bass_kernel_spmd(nc, in_maps, core_ids=list(range(NC)),
                               tmpdir=tdir if tdir else None)
    global LAST_RESULT
    LAST_RESULT = res

    out = np.zeros((B, T), np.float32)
    for c in range(NC):
        pr = res.results[c]["pred"].reshape(BL, T)   # tok = b*T + t
        out[c * BL:(c + 1) * BL, :] = pr
    return out


if __name__ == "__main__":
    import time
    rng = np.random.default_rng(0)
    s = 0.05
    ins = {
        "item_seq": rng.integers(0, NUM_ITEM, (B, T)),
        "correct_seq": rng.integers(0, 2, (B, T)),
        "k_emb": (rng.standard_normal((NUM_ITEM, DK)) * s).astype(np.float32),
        "v_emb": (rng.standard_normal((2 * NUM_ITEM, DK)) * s).astype(np.float32),
        "Mk": (rng.standard_normal((DV, DK)) * s).astype(np.float32),
        "Mv0": (rng.standard_normal((DV, DK)) * s).astype(np.float32),
        "e_W": (rng.standard_normal((DK, DK)) * s).astype(np.float32),
        "e_b": np.zeros(DK, np.float32),
        "a_W": (rng.standard_normal((DK, DK)) * s).astype(np.float32),
        "a_b": np.zeros(DK, np.float32),
        "f_W": (rng.standard_normal((DK, 2 * DK)) * s).astype(np.float32),
        "f_b": np.zeros(DK, np.float32),
        "p_W": (rng.standard_normal((1, DK)) * s).astype(np.float32),
        "p_b": np.zeros(1, np.float32),
    }
    t0 = time.time()
    out = kernel(**ins)
    print("kernel wall:", time.time() - t0)

    k = ins["k_emb"][ins["item_seq"]]
    v = ins["v_emb"][ins["item_seq"] + NUM_ITEM * ins["correct_seq"]]
    logits = k @ ins["Mk"].T
    w = np.exp(logits - logits.max(-1, keepdims=True))
    w /= w.sum(-1, keepdims=True)
    e = 1 / (1 + np.exp(-(v @ ins["e_W"].T + ins["e_b"])))
    a = np.tanh(v @ ins["a_W"].T + ins["a_b"])
    M = np.broadcast_to(ins["Mv0"][None], (B, DV, DK)).copy()
    reads = np.zeros((B, T, DK), np.float32)
    for t in range(T):
        reads[:, t] = np.einsum("bv,bvk->bk", w[:, t], M)
        M = M * (1 - w[:, t][:, :, None] * e[:, t][:, None, :]) \
            + w[:, t][:, :, None] * a[:, t][:, None, :]
    f = np.tanh(np.concatenate([reads, k], -1) @ ins["f_W"].T + ins["f_b"])
    ref = 1 / (1 + np.exp(-(f @ ins["p_W"].T + ins["p_b"])))[:, :, 0]
    err = np.abs(out - ref)
    print("max abs err:", err.max(), " rel:", err.max() / np.abs(ref).max())


# revision 28
# speedup vs baseline: 1.3977x; 1.3977x over previous
"""DKVMN forward kernel for 8 Trainium2 NeuronCores (Bass/Tile) — v2.

Design (replaces the tensor_tensor_scan/slot design):
 - Data-parallel over batch: core c handles batches [c*32, (c+1)*32).
 - State layout: p = b_local*4 + kq (128 partitions), free = (k2=64, v=128);
   M[p, k2*128+v] = M_state[b][v, kq*64+k2], bf16.
 - Device-built DRAM table DUtab[(x, tab, q)] of quarter-rows (8192 core
   elems + 128 tail): D = 1 - w[item(x)] (x) e[x] (outer, k-major/v-inner),
   U = w (x) a.  D-rows carry w[item] in the tail for the read-multiply.
 - Per chunk (C=2 steps): two dma_gathers (16.6KB rows, ~8.5MB/chunk) fetch
   D/U/w for 2 steps; per step 4 packed DVE ops (all 2x-mode eligible):
     RT = M * w_bcast ; r = reduce_v(RT) ; M = M * D ; M = M + U
 - reads stored bf16 token-major [tok, k]; head uses dma_start_transpose
   (xbar) to get [k, tok] tiles, PE matmuls for fW1@r and p_W.f, KF via
   transposed dma_gather.  No strided 4-byte DMA anywhere.
"""
import os
import sys
import numpy as np
import ml_dtypes

sys.path.insert(0, '/opt/trn_rl_repo')

import concourse.bass as bass          # noqa: E402
import concourse.bacc as bacc          # noqa: E402
import concourse.mybir as mybir        # noqa: E402
from concourse.tile import TileContext # noqa: E402
from concourse.bass_utils import run_bass_kernel_spmd  # noqa: E402

F32 = mybir.dt.float32
BF16 = mybir.dt.bfloat16
I16 = mybir.dt.int16
ALU = mybir.AluOpType
ACTF = mybir.ActivationFunctionType
BF = ml_dtypes.bfloat16

NUM_ITEM = 2000
DK = 256          # key dim
DV = 128          # memory slots (v)
B, T = 256, 512
NC = 8
BL = B // NC      # 32 local batches
KSUB = 4          # k quarters on partitions
K2 = DK // KSUB   # 64
P = BL * KSUB     # 128 partitions: p = kq*32 + b (kq-major)
SEG = K2 * DV     # 8192 state cells per partition
C = 2             # steps per chunk
NCH = T // C      # 256 chunks
NIT = 2048        # padded item count
NX = 4096         # padded x count (e/a tables)
NXR = 4000        # real x count (D/U tables)
ROW = SEG + DV    # 8320 elems per D quarter-row (core + w tail)
TOK = BL * T      # 16384 tokens per core
TH = TOK // 2     # head token-half

_cache = {}
LAST_RESULT = None  # BassKernelResults of the most recent run (for test harness)


def _wrap16(vals):
    """int index array [n] -> [128, n/16] wrapped-in-16 + replicated x8."""
    vals = np.asarray(vals, np.int64)
    n = len(vals)
    assert n % 16 == 0
    a = np.zeros((16, n // 16), np.int16)
    a[np.arange(n) % 16, np.arange(n) // 16] = vals
    return np.tile(a, (8, 1))


def build_program():
    nc = bacc.Bacc(None, target_bir_lowering=False, debug=False)

    # ---- external inputs (host-prepped) ----
    kT = nc.dram_tensor("kT", [DK, NIT], BF16, kind="ExternalInput")        # k_emb^T padded
    vT = nc.dram_tensor("vT", [DK, NX], BF16, kind="ExternalInput")         # v_emb^T padded
    MkT = nc.dram_tensor("MkT", [DK, DV], BF16, kind="ExternalInput")       # Mk^T
    eaWT = nc.dram_tensor("eaWT", [DK, 2 * DK], BF16, kind="ExternalInput")  # [e_W^T | a_W^T]
    fW2T = nc.dram_tensor("fW2T", [DK, DK], BF16, kind="ExternalInput")     # f_W[:,256:]^T
    fW1T = nc.dram_tensor("fW1T", [DK, DK], BF16, kind="ExternalInput")     # f_W[:,:256]^T
    onesf = nc.dram_tensor("onesf", [1, 128], F32, kind="ExternalInput")
    eab = nc.dram_tensor("eab", [1, 2 * DK], F32, kind="ExternalInput")     # [e_b | a_b]
    fbcol = nc.dram_tensor("fbcol", [128, 2], F32, kind="ExternalInput")    # f_b by ko-half
    pwcol = nc.dram_tensor("pwcol", [128, 2], BF16, kind="ExternalInput")   # p_W by ko-half
    pbt = nc.dram_tensor("pbt", [1, 1], F32, kind="ExternalInput")
    m0 = nc.dram_tensor("m0", [P, SEG], BF16, kind="ExternalInput")
    cidx = nc.dram_tensor("cidx", [P, NCH * 16], I16, kind="ExternalInput")
    wxidx = nc.dram_tensor("wxidx", [P, NX // 16], I16, kind="ExternalInput")
    kfidx = nc.dram_tensor("kfidx", [P, TOK // 16], I16, kind="ExternalInput")

    pred = nc.dram_tensor("pred", [1, TOK], F32, kind="ExternalOutput")

    # ---- DRAM scratch ----
    WtabD = nc.dram_tensor("WtabD", [NIT, DV], BF16)      # softmax rows (item)
    KFtabD = nc.dram_tensor("KFtabD", [NIT, DK], BF16)    # k_emb @ fW2^T rows
    Dtab = nc.dram_tensor("Dtab", [NXR * 4, ROW], BF16)   # (x, q): 1-w(x)e + w tail
    Utab = nc.dram_tensor("Utab", [NXR * 4, SEG], BF16)   # (x, q): w(x)a
    rT_d2 = nc.dram_tensor("rT_d2", [TOK, DK], BF16)      # reads, token-major

    with TileContext(nc) as tc:
        # ================= stage 1a: w / e / a / kf tables =================
        with tc.tile_pool(name="keep", bufs=1) as kp:
            ea_bf = kp.tile([128, 32, 2 * DK], BF16, tag="eabf")   # sig/tanh rows (x)
            wx_sb = kp.tile([128, 32, DV], BF16, tag="wx")         # w rows by x
            with (
                tc.tile_pool(name="wpool", bufs=1) as wp,
                tc.tile_pool(name="tpool", bufs=1) as tp,
                tc.tile_pool(name="pspool", bufs=2, space="PSUM") as pp,
            ):
                kT_s = [wp.tile([128, NIT], BF16, tag=f"kt{i}", name=f"kt{i}") for i in range(2)]
                vT_s = [wp.tile([128, NX], BF16, tag=f"vt{i}", name=f"vt{i}") for i in range(2)]
                MkT_s = [wp.tile([128, DV], BF16, tag=f"mk{i}", name=f"mk{i}") for i in range(2)]
                eaWT_s = [wp.tile([128, 2 * DK], BF16, tag=f"ea{i}", name=f"eaw{i}") for i in range(2)]
                fW2T_s = [wp.tile([128, DK], BF16, tag=f"f2{i}", name=f"f2{i}") for i in range(2)]
                onesf_s = wp.tile([1, 128], F32, tag="onf")
                eab_s = wp.tile([1, 2 * DK], F32, tag="eb")
                for i in range(2):
                    nc.sync.dma_start(kT_s[i][:], kT[128 * i:128 * (i + 1), :])
                    nc.sync.dma_start(vT_s[i][:], vT[128 * i:128 * (i + 1), :])
                    nc.sync.dma_start(MkT_s[i][:], MkT[128 * i:128 * (i + 1), :])
                    nc.sync.dma_start(eaWT_s[i][:], eaWT[128 * i:128 * (i + 1), :])
                    nc.sync.dma_start(fW2T_s[i][:], fW2T[128 * i:128 * (i + 1), :])
                nc.sync.dma_start(onesf_s[:], onesf[:])
                nc.sync.dma_start(eab_s[:], eab[:])

                # --- softmax(k_emb @ Mk^T) rows -> WtabD (bf16) ---
                wexp = tp.tile([128, 16, DV], F32, tag="wexp")
                for it in range(16):
                    ps = pp.tile([128, DV], F32, tag="ps_w")
                    sl = slice(128 * it, 128 * (it + 1))
                    nc.tensor.matmul(out=ps[:], lhsT=kT_s[0][:, sl], rhs=MkT_s[0][:],
                                     start=True, stop=False)
                    nc.tensor.matmul(out=ps[:], lhsT=kT_s[1][:, sl], rhs=MkT_s[1][:],
                                     start=False, stop=True)
                    nc.scalar.activation(out=wexp[:, it, :], in_=ps[:], func=ACTF.Exp)
                zs = tp.tile([128, 16], F32, tag="zs")
                nc.vector.tensor_reduce(out=zs[:], in_=wexp[:], axis=mybir.AxisListType.X,
                                        op=ALU.add)
                zr = tp.tile([128, 16], F32, tag="zr")
                nc.vector.reciprocal(out=zr[:], in_=zs[:])
                wbf = tp.tile([128, 16, DV], BF16, tag="wbf")
                nc.vector.tensor_tensor(
                    out=wbf[:], in0=wexp[:],
                    in1=zr[:].unsqueeze(2).to_broadcast([128, 16, DV]), op=ALU.mult)
                nc.sync.dma_start(
                    WtabD[:].rearrange("(it p) v -> p it v", p=128), wbf[:])

                # --- sigmoid/tanh(v_emb @ [eW|aW]^T + [eb|ab]) rows (by x) ---
                for it in range(32):
                    ps = pp.tile([128, 2 * DK], F32, tag="ps_ea")
                    sl = slice(128 * it, 128 * (it + 1))
                    nc.tensor.matmul(out=ps[:], lhsT=vT_s[0][:, sl], rhs=eaWT_s[0][:],
                                     start=True, stop=False)
                    nc.tensor.matmul(out=ps[:], lhsT=vT_s[1][:, sl], rhs=eaWT_s[1][:],
                                     start=False, stop=False)
                    nc.tensor.matmul(out=ps[:], lhsT=onesf_s[:], rhs=eab_s[:],
                                     start=False, stop=True)
                    nc.scalar.activation(out=ea_bf[:, it, 0:DK], in_=ps[:, 0:DK],
                                         func=ACTF.Sigmoid)
                    nc.scalar.activation(out=ea_bf[:, it, DK:2 * DK], in_=ps[:, DK:2 * DK],
                                         func=ACTF.Tanh)

                # --- KFtabD: k_emb @ fW2^T (NO f_b; added later via ACT bias) ---
                kfbf = tp.tile([128, 16, DK], BF16, tag="kfbf")
                for it in range(16):
                    ps = pp.tile([128, DK], F32, tag="ps_kf")
                    sl = slice(128 * it, 128 * (it + 1))
                    nc.tensor.matmul(out=ps[:], lhsT=kT_s[0][:, sl], rhs=fW2T_s[0][:],
                                     start=True, stop=False)
                    nc.tensor.matmul(out=ps[:], lhsT=kT_s[1][:, sl], rhs=fW2T_s[1][:],
                                     start=False, stop=True)
                    nc.scalar.copy(out=kfbf[:, it, :], in_=ps[:])
                nc.sync.dma_start(
                    KFtabD[:].rearrange("(it p) c -> p it c", p=128), kfbf[:])

                # w rows re-fetched x-aligned (item(x) = x mod 2000)
                wxi_s = tp.tile([P, NX // 16], I16, tag="wxi")
                nc.sync.dma_start(wxi_s[:], wxidx[:])
                for gq in range(4):
                    nc.gpsimd.dma_gather(
                        wx_sb[:, 8 * gq:8 * (gq + 1), :], WtabD[:],
                        wxi_s[:, 64 * gq:64 * (gq + 1)], NX // 4, NX // 4, DV)

            # ================= stage 1b: D/U table build =================
            D4 = Dtab[:].rearrange("(x q) e -> x (q e)", q=KSUB)
            U4 = Utab[:].rearrange("(x q) e -> x q e", q=KSUB)
            with tc.tile_pool(name="bpool", bufs=2) as bp:
                for g in range(32):
                    npart = min(128, NXR - 128 * g)   # last tile covers 32 x's
                    xsl = slice(128 * g, 128 * g + npart)
                    for tab in range(2):
                        BT = bp.tile([128, KSUB, ROW], BF16, tag="bt")
                        core = BT[:, :, 0:SEG].rearrange(
                            "p q (k2 v) -> p q k2 v", v=DV)      # [p, 4, 64, 128]
                        col = slice(0, DK) if tab == 0 else slice(DK, 2 * DK)
                        e4 = ea_bf[:, g, col].rearrange(
                            "p (q k2) -> p q k2", q=KSUB).unsqueeze(3) \
                            .to_broadcast([128, KSUB, K2, DV])
                        w4 = wx_sb[:, g, :].unsqueeze(1).unsqueeze(2) \
                            .to_broadcast([128, KSUB, K2, DV])
                        nc.vector.tensor_tensor(out=core, in0=e4, in1=w4,
                                                op=ALU.mult)
                        if tab == 0:
                            nc.vector.tensor_scalar(
                                out=core, in0=core, scalar1=-1.0, scalar2=1.0,
                                op0=ALU.mult, op1=ALU.add)
                            nc.vector.tensor_copy(
                                out=BT[:, :, SEG:ROW],
                                in_=wx_sb[:, g, :].unsqueeze(1).to_broadcast(
                                    [128, KSUB, DV]))
                            nc.sync.dma_start(
                                D4[xsl, :],
                                BT[0:npart].rearrange("p q e -> p (q e)"))
                        else:
                            nc.sync.dma_start(
                                U4[xsl], BT[0:npart, :, 0:SEG])

        # ================= stage 2: recurrence =================
        with (
            tc.tile_pool(name="scst", bufs=1) as st,
            tc.tile_pool(name="scg", bufs=2) as sg,
            tc.tile_pool(name="rp", bufs=2) as rp,
        ):
            Mt = st.tile([P, SEG], BF16, tag="M")
            RT = st.tile([P, SEG], BF16, tag="RT")
            cidx_sb = st.tile([P, NCH, 16], I16, tag="cix")
            nc.sync.dma_start(Mt[:], m0[:])
            nc.sync.dma_start(cidx_sb[:].rearrange("p a b -> p (a b)"), cidx[:])

            M3 = Mt[:].rearrange("p (k v) -> p k v", v=DV)
            RT3 = RT[:].rearrange("p (k v) -> p k v", v=DV)
            R2 = rT_d2[:].rearrange("(b t) k -> b t k", b=BL)

            def fetch(ch):
                Dg = sg.tile([P, C, ROW], BF16, tag="dg", name="Dg")
                Ug = sg.tile([P, C, SEG], BF16, tag="ug", name="Ug")
                nc.gpsimd.dma_gather(Dg[:], Dtab[:],
                                     cidx_sb[:, ch, :], C * P, C * P, ROW)
                nc.gpsimd.dma_gather(Ug[:], Utab[:],
                                     cidx_sb[:, ch, :], C * P, C * P, SEG)
                return Dg, Ug

            nxt = fetch(0)
            for ch in range(NCH):
                Dg, Ug = nxt
                if ch + 1 < NCH:
                    nxt = fetch(ch + 1)
                rTf = rp.tile([P, C, K2], F32, tag="rtf")
                rTb = rp.tile([P, C, K2], BF16, tag="rtb")
                for s in range(C):
                    w_ap = Dg[:, s, SEG:ROW].unsqueeze(1).to_broadcast([P, K2, DV])
                    nc.vector.tensor_tensor(out=RT3, in0=M3, in1=w_ap, op=ALU.mult)
                    nc.vector.tensor_reduce(out=rTf[:, s, :], in_=RT3,
                                            axis=mybir.AxisListType.X, op=ALU.add)
                    nc.vector.tensor_tensor(out=Mt[:], in0=Mt[:],
                                            in1=Dg[:, s, 0:SEG], op=ALU.mult)
                    nc.vector.tensor_tensor(out=Mt[:], in0=Mt[:],
                                            in1=Ug[:, s, :], op=ALU.add)
                nc.scalar.copy(out=rTb[:], in_=rTf[:])
                for kq in range(KSUB):
                    nc.sync.dma_start(
                        R2[:, C * ch:C * (ch + 1), K2 * kq:K2 * (kq + 1)],
                        rTb[BL * kq:BL * (kq + 1), :, :])

        # ================= stage 3: head =================
        with (
            tc.tile_pool(name="hw", bufs=1) as hw,
            tc.tile_pool(name="hp", bufs=1) as hpool,
            tc.tile_pool(name="hps", bufs=4, space="PSUM") as hps,
            tc.tile_pool(name="hps2", bufs=4, space="PSUM") as hps2,
        ):
            f1 = [hw.tile([128, DK], BF16, tag=f"f1{h}", name=f"f1{h}") for h in range(2)]
            for h in range(2):
                nc.sync.dma_start(f1[h][:], fW1T[128 * h:128 * (h + 1), :])
            pw_s = hw.tile([128, 2], BF16, tag="pw")
            fb_s = hw.tile([128, 2], F32, tag="fb")
            pb_s = hw.tile([1, 1], F32, tag="pb")
            kfi_s = hw.tile([P, TOK // 16], I16, tag="kfi")
            pred_sb = hw.tile([1, TOK], F32, tag="prd")
            nc.sync.dma_start(pw_s[:], pwcol[:])
            nc.sync.dma_start(fb_s[:], fbcol[:])
            nc.sync.dma_start(pb_s[:], pbt[:])
            nc.sync.dma_start(kfi_s[:], kfidx[:])

            for th in range(2):
                toks = slice(th * TH, (th + 1) * TH)
                rq = [hpool.tile([128, TH], BF16, tag=f"rq{h}", name=f"rq{h}") for h in range(2)]
                for h in range(2):
                    nc.sync.dma_start_transpose(
                        rq[h][:], rT_d2[toks, 128 * h:128 * (h + 1)])
                kfT = hpool.tile([128, TH // 512, 2, 512], BF16, tag="kft")
                for gq in range(TH // 512):
                    nc.gpsimd.dma_gather(
                        kfT[:, gq, :, :], KFtabD[:],
                        kfi_s[:, th * (TH // 16) + 32 * gq:
                              th * (TH // 16) + 32 * (gq + 1)],
                        512, 512, DK, transpose=True)
                fq = hpool.tile([128, 2, TH], BF16, tag="fq")
                for m in range(2):
                    for n in range(TH // 512):
                        nsl = slice(512 * n, 512 * (n + 1))
                        ps = hps.tile([128, 512], F32, tag="psg")
                        nc.tensor.matmul(out=ps[:], lhsT=f1[0][:, 128 * m:128 * (m + 1)],
                                         rhs=rq[0][:, nsl], start=True, stop=False)
                        nc.tensor.matmul(out=ps[:], lhsT=f1[1][:, 128 * m:128 * (m + 1)],
                                         rhs=rq[1][:, nsl], start=False, stop=True)
                        nc.vector.tensor_tensor(out=fq[:, m, nsl], in0=ps[:],
                                                in1=kfT[:, n, m, :], op=ALU.add)
                    nc.scalar.activation(out=fq[:, m, :], in_=fq[:, m, :],
                                         func=ACTF.Tanh, bias=fb_s[:, m:m + 1])
                for n in range(TH // 512):
                    nsl = slice(512 * n, 512 * (n + 1))
                    ps2 = hps2.tile([1, 512], F32, tag="psp")
                    nc.tensor.matmul(out=ps2[:], lhsT=pw_s[:, 0:1],
                                     rhs=fq[:, 0, nsl], start=True, stop=False)
                    nc.tensor.matmul(out=ps2[:], lhsT=pw_s[:, 1:2],
                                     rhs=fq[:, 1, nsl], start=False, stop=True)
                    nc.scalar.activation(out=pred_sb[:, th * TH + 512 * n:
                                                     th * TH + 512 * (n + 1)],
                                         in_=ps2[:], func=ACTF.Sigmoid,
                                         bias=pb_s[:])
            nc.sync.dma_start(pred[:], pred_sb[:])

    nc.finalize()
    return nc


def _host_shared(k_emb, v_emb, Mk, Mv0, e_W, e_b, a_W, a_b, f_W, f_b, p_W, p_b):
    pad_k = np.zeros((NIT, DK), np.float32)
    pad_k[:NUM_ITEM] = k_emb
    pad_v = np.zeros((NX, DK), np.float32)
    pad_v[:2 * NUM_ITEM] = v_emb
    fbcol = np.zeros((128, 2), np.float32)
    fbcol[:, 0] = f_b[:128]
    fbcol[:, 1] = f_b[128:]
    pwcol = np.zeros((128, 2), np.float32)
    pwcol[:, 0] = p_W[0, :128]
    pwcol[:, 1] = p_W[0, 128:]
    # m0[p, k2*128+v] = Mv0[v, kq*64+k2] with p = kq*32 + b
    kq = np.arange(P) // BL
    k2i, vi = np.meshgrid(np.arange(K2), np.arange(DV), indexing="ij")
    m0 = Mv0.T[(kq[:, None, None] * K2 + k2i[None]), vi[None]].reshape(P, SEG)
    # wxidx: item(x) for x in [0, NX)
    items = np.zeros(NX, np.int64)
    items[:2 * NUM_ITEM] = np.arange(2 * NUM_ITEM) % NUM_ITEM
    return {
        "kT": np.ascontiguousarray(pad_k.T).astype(BF),
        "vT": np.ascontiguousarray(pad_v.T).astype(BF),
        "MkT": np.ascontiguousarray(Mk.T).astype(BF),
        "eaWT": np.ascontiguousarray(np.concatenate([e_W.T, a_W.T], axis=1)).astype(BF),
        "fW2T": np.ascontiguousarray(f_W[:, DK:].T).astype(BF),
        "fW1T": np.ascontiguousarray(f_W[:, :DK].T).astype(BF),
        "onesf": np.ones((1, 128), np.float32),
        "eab": np.concatenate([e_b, a_b])[None, :].astype(np.float32),
        "fbcol": fbcol,
        "pwcol": pwcol.astype(BF),
        "pbt": np.array([[float(p_b[0])]], np.float32),
        "m0": m0.astype(BF),
        "wxidx": _wrap16(items),
    }


def _host_core(item_c, x_c):
    """Per-core index tensors. item_c/x_c: [BL, T] int64. p = kq*32 + b."""
    b = np.arange(P) % BL
    kq = np.arange(P) // BL
    xp = x_c[b[None, :], np.arange(T)[:, None]]       # [T, P]
    rows = xp * KSUB + kq[None, :]                     # [T, P]
    cidx = np.zeros((P, NCH, 16), np.int16)
    for ch in range(NCH):
        cidx[:, ch, :] = _wrap16(rows[C * ch:C * (ch + 1), :].reshape(-1))
    return {
        "cidx": cidx.reshape(P, NCH * 16),
        "kfidx": _wrap16(item_c.reshape(-1)),
    }


def kernel(**inputs):
    inputs = {k: np.asarray(v) for k, v in inputs.items()}
    item = inputs["item_seq"].astype(np.int64)
    corr = inputs["correct_seq"].astype(np.int64)
    x = item + NUM_ITEM * corr

    if "nc" not in _cache:
        _cache["nc"] = build_program()
    nc = _cache["nc"]

    shared = _host_shared(
        inputs["k_emb"].astype(np.float32), inputs["v_emb"].astype(np.float32),
        inputs["Mk"].astype(np.float32), inputs["Mv0"].astype(np.float32),
        inputs["e_W"].astype(np.float32), inputs["e_b"].astype(np.float32),
        inputs["a_W"].astype(np.float32), inputs["a_b"].astype(np.float32),
        inputs["f_W"].astype(np.float32), inputs["f_b"].astype(np.float32),
        inputs["p_W"].astype(np.float32), inputs["p_b"].astype(np.float32))

    in_maps = []
    for c in range(NC):
        sl = slice(c * BL, (c + 1) * BL)
        m = dict(shared)
        m.update(_host_core(item[sl], x[sl]))
        in_maps.append(m)

    tdir = os.environ.get("BASS_KERNEL_TRACE_DIR")
    res = run_bass_kernel_spmd(nc, in_maps, core_ids=list(range(NC)),
                               tmpdir=tdir if tdir else None)
    global LAST_RESULT
    LAST_RESULT = res

    out = np.zeros((B, T), np.float32)
    for c in range(NC):
        pr = res.results[c]["pred"].reshape(BL, T)   # tok = b*T + t
        out[c * BL:(c + 1) * BL, :] = pr
    return out


if __name__ == "__main__":
    import time
    rng = np.random.default_rng(0)
    s = 0.05
    ins = {
        "item_seq": rng.integers(0, NUM_ITEM, (B, T)),
        "correct_seq": rng.integers(0, 2, (B, T)),
        "k_emb": (rng.standard_normal((NUM_ITEM, DK)) * s).astype(np.float32),
        "v_emb": (rng.standard_normal((2 * NUM_ITEM, DK)) * s).astype(np.float32),
        "Mk": (rng.standard_normal((DV, DK)) * s).astype(np.float32),
        "Mv0": (rng.standard_normal((DV, DK)) * s).astype(np.float32),
        "e_W": (rng.standard_normal((DK, DK)) * s).astype(np.float32),
        "e_b": np.zeros(DK, np.float32),
        "a_W": (rng.standard_normal((DK, DK)) * s).astype(np.float32),
        "a_b": np.zeros(DK, np.float32),
        "f_W": (rng.standard_normal((DK, 2 * DK)) * s).astype(np.float32),
        "f_b": np.zeros(DK, np.float32),
        "p_W": (rng.standard_normal((1, DK)) * s).astype(np.float32),
        "p_b": np.zeros(1, np.float32),
    }
    t0 = time.time()
    out = kernel(**ins)
    print("kernel wall:", time.time() - t0)

    k = ins["k_emb"][ins["item_seq"]]
    v = ins["v_emb"][ins["item_seq"] + NUM_ITEM * ins["correct_seq"]]
    logits = k @ ins["Mk"].T
    w = np.exp(logits - logits.max(-1, keepdims=True))
    w /= w.sum(-1, keepdims=True)
    e = 1 / (1 + np.exp(-(v @ ins["e_W"].T + ins["e_b"])))
    a = np.tanh(v @ ins["a_W"].T + ins["a_b"])
    M = np.broadcast_to(ins["Mv0"][None], (B, DV, DK)).copy()
    reads = np.zeros((B, T, DK), np.float32)
    for t in range(T):
        reads[:, t] = np.einsum("bv,bvk->bk", w[:, t], M)
        M = M * (1 - w[:, t][:, :, None] * e[:, t][:, None, :]) \
            + w[:, t][:, :, None] * a[:, t][:, None, :]
    f = np.tanh(np.concatenate([reads, k], -1) @ ins["f_W"].T + ins["f_b"])
    ref = 1 / (1 + np.exp(-(f @ ins["p_W"].T + ins["p_b"])))[:, :, 0]
    err = np.abs(out - ref)
    print("max abs err:", err.max(), " rel:", err.max() / np.abs(ref).max())
